# revision 11
# baseline (speedup 1.0000x reference)
"""Trainium2 Bass kernel for nn_AngleModel (2-layer TransformerConv GNN).

Self-contained: host-side graph preprocessing + Bass/Tile program + SPMD run
on 8 NeuronCores via concourse. See design notes inline.

Strategy:
  - Nodes assigned to cores by original-id quarter (quarter q -> cores 2q,2q+1,
    round-robin by degree), so position-space quarters == original quarters.
  - Per-core nodes sorted by per-window degree (lexicographic) -> ELL slot
    format with tiny padding. Per (tile, window) slots gathered from the k|v
    table with dma_gather (int16 idx, 4 overlapping 32768-row windows).
  - All per-edge math is node-aligned elementwise on the Vector engine; the
    segment softmax becomes a free-dim reduce. No segment-max needed (alphas
    are O(1)); denominator fused: out = (sum w*v + (sum w*attr)*We)/(den+eps).
  - Layer 1 -> AllGather of transposed h -> layer 2 -> fc/normalize/masks.
"""
import sys
import types
import numpy as np

# ---- shim the missing antenv.axon_hooks so NTFF profiling works ----
try:
    import antenv
    if "antenv.axon_hooks" not in sys.modules:
        _mod = types.ModuleType("antenv.axon_hooks")
        _HOOK = [None]
        _mod.set_axon_ntff_profile_hook = lambda h: _HOOK.__setitem__(0, h)
        _mod.get_axon_ntff_profile_hook = lambda: _HOOK[0]
        sys.modules["antenv.axon_hooks"] = _mod
        antenv.axon_hooks = _mod
        try:
            from trn_agent_boot.trn_boot import _ntff_profile_via_ctypes
            _mod.set_axon_ntff_profile_hook(
                _ntff_profile_via_ctypes('/opt/axon/libaxon_pjrt.so'))
        except Exception:
            pass
except Exception:
    pass

import concourse.bass as bass
import concourse.tile as tile
from concourse import bacc, mybir
from concourse.bass import AP
from concourse.bass_utils import run_bass_kernel_spmd
from concourse.alu_op_type import AluOpType

P = 128
NCORES = 8
F = 64                      # feat dim
KVW = 2 * F                 # k|v row width (128 f32 = 512B)
MAXIDX = 1024               # max idxs per dma_gather call
FP = mybir.dt.float32
I16 = mybir.dt.int16
AX = mybir.AxisListType.X
Act = mybir.ActivationFunctionType


# ----------------------------------------------------------------------------
# Host-side preprocessing
# ----------------------------------------------------------------------------

def _build_host(x, edge_index, edge_attr):
    N = x.shape[0]
    src = edge_index[0].astype(np.int64)
    dst = edge_index[1].astype(np.int64)
    ea = edge_attr[:, 0].astype(np.float32)

    NQ = 4
    QS = (N + NQ - 1) // NQ                  # original-id quarter size (25000)
    NLOC = QS // 2                            # nodes per core (12500)
    NLOCP = ((NLOC + P - 1) // P) * P         # padded per-core (12544)
    NT = NLOCP // P                           # tiles per core (98)
    NPOS = NCORES * NLOCP                     # padded position space (100352)
    WIN = 32768
    WBASE = [w * NLOCP * 2 for w in range(NQ)]   # window base rows (25088*w)

    deg = np.bincount(dst, minlength=N)

    # --- window assignment: primary quarter (always valid in position space) ---
    w_edge = src // QS

    dw = np.zeros((N, NQ), np.int64)
    np.add.at(dw.reshape(-1), dst * NQ + w_edge, 1)

    # --- node -> (core, ploc): quarter q -> cores 2q/2q+1 ---
    core_of = np.empty(N, np.int64)
    ploc_of = np.empty(N, np.int64)
    for q in range(NQ):
        ids = np.arange(q * QS, min((q + 1) * QS, N))
        o = ids[np.argsort(-deg[ids], kind="stable")]
        core_of[o] = 2 * q + (np.arange(len(o)) % 2)
        ploc_of[o] = np.arange(len(o)) // 2
    # re-sort within core by (d0,d1,d2,d3) desc for tight ELL tiles
    for c in range(NCORES):
        ids = np.where(core_of == c)[0]
        key = np.lexsort((-dw[ids, 3], -dw[ids, 2], -dw[ids, 1], -dw[ids, 0]))
        ids = ids[key]
        ploc_of[ids] = np.arange(len(ids))
    pos_of = core_of * NLOCP + ploc_of        # padded position space

    # validate window coverage on positions; fix any violators to primary win
    srcpos = pos_of[src]
    off = srcpos - np.take(WBASE, w_edge)
    bad = (off < 0) | (off >= WIN)
    if bad.any():
        w_edge[bad] = srcpos[bad] // (2 * NLOCP)
        off = srcpos - np.take(WBASE, w_edge)
        assert ((off >= 0) & (off < WIN)).all()

    # --- per-(core,tile,window) slot counts, uniform across cores ---
    dwp = np.zeros((NCORES, NLOCP, NQ), np.int64)
    dwp[core_of, ploc_of] = dw
    per_tile = dwp.reshape(NCORES, NT, P, NQ)
    Dw = per_tile.max(axis=(0, 2))            # [NT, NQ]
    # split any window block into <=8-slot (1024 idx) gather calls later
    for _t in range(NT):
        if Dw[_t].sum() == 0:
            Dw[_t, 0] = 1
    Dtot = Dw.sum(1)                          # [NT]
    woff = np.concatenate([np.zeros((NT, 1), np.int64), np.cumsum(Dw, 1)], 1)
    tile_base = np.concatenate([[0], np.cumsum(Dtot)])  # slot offsets per tile
    SLOTS = int(tile_base[-1])                # slots per core (uniform)

    # --- per-core slot arrays ---
    # layout: attr/valid node-major [tile][P, Dtot_t]; idx slot-major wrapped
    cores = []
    # per-edge slot index within (dst, window): cumcount
    ek = (core_of[dst] * NLOCP + ploc_of[dst]) * NQ + w_edge
    eorder = np.argsort(ek, kind="stable")
    ek_s = ek[eorder]
    # cumcount via unique segments
    _, seg_start, seg_cnt = np.unique(ek_s, return_index=True,
                                      return_counts=True)
    jj = np.arange(len(ek_s)) - np.repeat(seg_start, seg_cnt)

    e_src = srcpos[eorder]
    e_off = off[eorder]
    e_att = ea[eorder]
    e_core = core_of[dst[eorder]]
    e_ploc = ploc_of[dst[eorder]]
    e_w = w_edge[eorder]
    e_tile = e_ploc // P
    e_part = e_ploc % P
    e_slot = woff[e_tile, e_w] + jj           # slot index within tile row

    IDXW = 8 * int(Dtot.max())                # int16 words per partition-row
    for c in range(NCORES):
        m = e_core == c
        t_, p_, s_, o_, a_ = (e_tile[m], e_part[m], e_slot[m], e_off[m],
                              e_att[m])
        # per tile contiguous [P, Dtot_t]
        fl_base_t = (tile_base * P)
        fl = fl_base_t[t_] + p_ * Dtot[t_] + s_
        attr_fl = np.zeros(int(P * tile_base[-1]), np.float32)
        val_fl = np.zeros(int(P * tile_base[-1]), np.float32)
        attr_fl[fl] = a_
        val_fl[fl] = 1.0
        # idx: per (tile, w) slot-major wrapped int16
        idx_fl = np.zeros((NT, P, IDXW), np.uint16)
        i_lin = (s_ - woff[t_, e_w[m]]) * P + p_     # within-window flat idx
        base_w = (woff[t_, e_w[m]] * 8)
        # wrapped position: [i%16, i//16] within the window's block
        rows = i_lin % 16
        colw = base_w + i_lin // 16
        blk = np.zeros((NT, 16, IDXW), np.uint16)
        blk[t_, rows, colw] = o_.astype(np.uint16)
        idx_fl[:, :, :] = np.tile(blk, (1, 8, 1))
        cores.append(dict(attr=attr_fl, valid=val_fl, idx=idx_fl.view(np.int16)))

    H = dict(N=N, NQ=NQ, QS=QS, NLOC=NLOC, NLOCP=NLOCP, NT=NT, NPOS=NPOS,
             WIN=WIN, WBASE=WBASE, Dw=Dw, Dtot=Dtot, woff=woff,
             tile_base=tile_base, SLOTS=SLOTS, IDXW=IDXW,
             pos_of=pos_of, core_of=core_of, ploc_of=ploc_of, cores=cores)
    return H


def _pack_weights(params, H, x):
    p = {k: np.asarray(v) for k, v in params.items()}
    N = H["N"]
    NPOS, NLOCP, NT = H["NPOS"], H["NLOCP"], H["NT"]
    pos_of = H["pos_of"]

    # position-ordered x, transposed+augmented per global tile [NPOS/P, 7, P]
    xp = np.zeros((NPOS, 6), np.float32)
    xp[pos_of] = x
    GT = NPOS // P
    xT = np.zeros((GT, 7, P), np.float32)
    xT[:, :6, :] = xp.reshape(GT, P, 6).transpose(0, 2, 1)
    xT[:, 6, :] = 1.0

    def aug(Ws, bs):  # [K,out] list -> [K+1, sum_out]
        W = np.concatenate(Ws, 1)
        b = np.concatenate(bs)
        return np.concatenate([W, b[None, :]], 0).astype(np.float32)

    We1 = p['We1'][0]          # [64]
    We2 = p['We2'][0]
    w1kv = aug([p['Wk1'], p['Wv1']], [p['bk1'], p['bv1']])       # [7,128]
    w1qs = aug([p['Wq1'], p['Ws1'], (p['Wq1'] @ We1)[:, None]],
               [p['bq1'], p['bs1'], (p['bq1'] @ We1)[None]])     # [7,129]
    w2kv = aug([p['Wk2'], p['Wv2']], [p['bk2'], p['bv2']])       # [65,128]
    w2qs = aug([p['Wq2'], p['Ws2'], (p['Wq2'] @ We2)[:, None]],
               [p['bq2'], p['bs2'], (p['bq2'] @ We2)[None]])     # [65,129]
    weR = np.concatenate([np.tile(We1, (P, 1)), np.tile(We2, (P, 1))],
                         1)                                      # [P, 128]
    wfc = np.concatenate([np.tile(p['Wfc'][:, c], (P, 1))
                          for c in range(3)], 1)                 # [P, 192]
    bfc = p['bfc'].astype(np.float32)

    xmask = xp[:, [3, 5]].reshape(NCORES, NLOCP, 2).astype(np.float32)
    return dict(xT=xT, w1kv=w1kv, w1qs=w1qs, w2kv=w2kv, w2qs=w2qs,
                weR=weR.astype(np.float32), wfc=wfc.astype(np.float32),
                bfc=bfc, xmask=xmask)


# ----------------------------------------------------------------------------
# Device program
# ----------------------------------------------------------------------------

def _build_program(H):
    import os as _os
    MAXPH = int(_os.environ.get("KERNEL_MAXPH", "5"))
    BPART = int(_os.environ.get("KERNEL_BPART", "3"))
    NT, SLOTS, IDXW = H["NT"], H["SLOTS"], H["IDXW"]
    NPOS, NLOCP = H["NPOS"], H["NLOCP"]
    GT = NPOS // P
    Dw, Dtot, woff, tile_base = H["Dw"], H["Dtot"], H["woff"], H["tile_base"]
    WBASE, WIN = H["WBASE"], H["WIN"]

    nc = bacc.Bacc("TRN2", target_bir_lowering=False, debug=False,
                   num_devices=NCORES, num_swdge_queues=4)

    # inputs
    xT = nc.dram_tensor("xT", [GT, 7, P], FP, kind="ExternalInput")
    xTl = nc.dram_tensor("xTl", [NT, 7, P], FP, kind="ExternalInput")
    w1kv = nc.dram_tensor("w1kv", [7, 128], FP, kind="ExternalInput")
    w1qs = nc.dram_tensor("w1qs", [7, 129], FP, kind="ExternalInput")
    w2kv = nc.dram_tensor("w2kv", [65, 128], FP, kind="ExternalInput")
    w2qs = nc.dram_tensor("w2qs", [65, 129], FP, kind="ExternalInput")
    weR = nc.dram_tensor("weR", [P, 2 * F], FP, kind="ExternalInput")
    wfc = nc.dram_tensor("wfc", [P, 3 * F], FP, kind="ExternalInput")
    bfc_in = nc.dram_tensor("bfc", [P, 3], FP, kind="ExternalInput")
    attr_in = nc.dram_tensor("attr", [P * int(tile_base[-1])], FP,
                             kind="ExternalInput")
    valid_in = nc.dram_tensor("valid", [P * int(tile_base[-1])], FP,
                              kind="ExternalInput")
    idx_in = nc.dram_tensor("idx", [NT, P, IDXW], I16, kind="ExternalInput")
    xmask = nc.dram_tensor("xmask", [NLOCP, 2], FP, kind="ExternalInput")

    # outputs
    out3 = nc.dram_tensor("out3", [NLOCP, 3], FP, kind="ExternalOutput")

    # internal DRAM
    kv1 = nc.dram_tensor("kv1", [NPOS, KVW], FP)
    kv2 = nc.dram_tensor("kv2", [NPOS, KVW], FP)
    qs1 = nc.dram_tensor("qs1", [NT, P, 129], FP)
    qs2 = nc.dram_tensor("qs2", [NT, P, 129], FP)
    hTl = nc.dram_tensor("hTl", [NT, F, P], FP)
    hTf = nc.dram_tensor("hTf", [NCORES * NT, F, P], FP, addr_space="Shared")

    from concourse.masks import make_identity
    from contextlib import ExitStack

    with tile.TileContext(nc) as tc, ExitStack() as _ctx:
        cst = _ctx.enter_context(tc.tile_pool(name="cst", bufs=1))
        w1kv_t = cst.tile([7, 128], FP); nc.sync.dma_start(out=w1kv_t[:], in_=w1kv[:, :])
        w1qs_t = cst.tile([7, 129], FP); nc.sync.dma_start(out=w1qs_t[:], in_=w1qs[:, :])
        w2kv_t = cst.tile([65, 128], FP); nc.sync.dma_start(out=w2kv_t[:], in_=w2kv[:, :])
        w2qs_t = cst.tile([65, 129], FP); nc.sync.dma_start(out=w2qs_t[:], in_=w2qs[:, :])
        weR_t = cst.tile([P, 2 * F], FP)
        nc.sync.dma_start(out=weR_t[:], in_=weR[:, :])
        wfc_t = cst.tile([P, 3 * F], FP); nc.sync.dma_start(out=wfc_t[:], in_=wfc[:, :])
        bfc_t = cst.tile([P, 3], FP); nc.sync.dma_start(out=bfc_t[:], in_=bfc_in[:, :])
        ident = cst.tile([P, P], FP)
        make_identity(nc, ident[:])

        # ---------------- phase A: layer-1 projections ----------------
        with tc.tile_pool(name="pa_x", bufs=3) as pa_x, \
             tc.tile_pool(name="pa_ps", bufs=2, space="PSUM") as pa_ps, \
             tc.tile_pool(name="pa_sb", bufs=3) as pa_sb:
            for g in range(GT):
                xt = pa_x.tile([7, P], FP)
                nc.sync.dma_start(out=xt[:], in_=xT[g, :, :])
                ps = pa_ps.tile([P, 128], FP)
                nc.tensor.matmul(out=ps[:], lhsT=xt[:], rhs=w1kv_t[:],
                                 start=True, stop=True)
                sb = pa_sb.tile([P, 128], FP)
                nc.vector.tensor_copy(out=sb[:], in_=ps[:])
                nc.sync.dma_start(out=kv1[g * P:(g + 1) * P, :], in_=sb[:])
            for t in range(NT):
                xt = pa_x.tile([7, P], FP)
                nc.sync.dma_start(out=xt[:], in_=xTl[t, :, :])
                ps = pa_ps.tile([P, 129], FP, tag="ps_qs")
                nc.tensor.matmul(out=ps[:], lhsT=xt[:], rhs=w1qs_t[:],
                                 start=True, stop=True)
                sb = pa_sb.tile([P, 129], FP, tag="sb_qs")
                nc.vector.tensor_copy(out=sb[:], in_=ps[:])
                nc.sync.dma_start(out=qs1[t, :, :], in_=sb[:])
        tc.strict_bb_all_engine_barrier()

        # ---------------- edge phase (shared for both layers) ----------------
        pool_dma_ctr = [0]

        def edge_phase(kvtab, qs, layer):
            pools = {}
            with tc.tile_pool(name=f"ep{layer}_meta", bufs=3) as meta, \
                 tc.tile_pool(name=f"ep{layer}_kv", bufs=2) as kvp, \
                 tc.tile_pool(name=f"ep{layer}_wk", bufs=2) as wk, \
                 tc.tile_pool(name=f"ep{layer}_sm", bufs=2) as sm, \
                 tc.tile_pool(name=f"ep{layer}_ps", bufs=2, space="PSUM") as psp:
                for t in range(NT):
                    D = int(Dtot[t])
                    if D == 0:
                        continue
                    qsw = meta.tile([P, 129], FP, tag="qsw")
                    nc.sync.dma_start(out=qsw[:], in_=qs[t, :, :])
                    at = meta.tile([P, D], FP, tag="at")
                    base = int(tile_base[t]) * P
                    nc.sync.dma_start(
                        out=at[:],
                        in_=AP(attr_in, base, [[D, P], [1, D]]))
                    va = meta.tile([P, D], FP, tag="va")
                    nc.sync.dma_start(
                        out=va[:],
                        in_=AP(valid_in, base, [[D, P], [1, D]]))
                    ixt = meta.tile([P, 8 * D], I16, tag="ixt")
                    nc.sync.dma_start(out=ixt[:], in_=idx_in[t, :, 0:8 * D])

                    kvt = kvp.tile([P, D * KVW], FP, tag="kvt")
                    for w in range(4):
                        dn = int(Dw[t, w])
                        o0 = int(woff[t, w])
                        s0 = 0
                        while s0 < dn:
                            sl = min(8, dn - s0)
                            L = sl * P
                            nc.gpsimd.dma_gather(
                                out_ap=AP(kvt.tensor, kvt[:].offset
                                          + (o0 + s0) * KVW,
                                          [kvt[:].ap[0], [KVW, sl], [1, KVW]]),
                                in_ap=kvtab[WBASE[w]:WBASE[w] + min(
                                    WIN, NPOS - WBASE[w]), :],
                                idxs_ap=ixt[:, 8 * (o0 + s0): 8 * (o0 + s0)
                                            + 8 * sl],
                                num_idxs=L,
                                num_idxs_reg=L,
                                elem_size=KVW,
                                queue_num=pool_dma_ctr[0] % 4,
                            )
                            pool_dma_ctr[0] += 1
                            s0 += sl
                    if BPART == 1:
                        continue

                    # alpha = sum_f k_slot*q  (then += attr*qwe; exp scale 1/8)
                    def _stop(n):
                        return BPART >= 20 and (BPART - 20) <= n
                    prod = wk.tile([P, D * F], FP, tag="prod")
                    kv_k = AP(kvt.tensor, kvt[:].offset,
                              [kvt[:].ap[0], [KVW, D], [1, F]])
                    q_b = AP(qsw.tensor, qsw[:].offset,
                             [qsw[:].ap[0], [0, D], [1, F]])
                    nc.vector.tensor_tensor(out=prod[:], in0=kv_k, in1=q_b,
                                            op=AluOpType.mult)
                    if _stop(1):
                        continue
                    alpha = sm.tile([P, D], FP, tag="alpha")
                    nc.vector.tensor_reduce(
                        out=alpha[:],
                        in_=AP(prod.tensor, prod[:].offset,
                               [prod[:].ap[0], [F, D], [1, F]]),
                        axis=AX, op=AluOpType.add)
                    if _stop(2):
                        continue
                    nc.vector.scalar_tensor_tensor(
                        out=alpha[:], in0=at[:], scalar=qsw[:, 128:129],
                        in1=alpha[:], op0=AluOpType.mult, op1=AluOpType.add)
                    if _stop(3):
                        continue
                    wt = sm.tile([P, D], FP, tag="wt")
                    nc.scalar.activation(out=wt[:], in_=alpha[:], func=Act.Exp,
                                         scale=0.125)
                    if _stop(4):
                        continue
                    den = sm.tile([P, 1], FP, tag="den")
                    wt2 = sm.tile([P, D], FP, tag="wt2")
                    nc.vector.tensor_tensor(out=wt2[:], in0=wt[:], in1=va[:],
                                            op=AluOpType.mult)
                    nc.vector.reduce_sum(out=den[:], in_=wt2[:], axis=AX)
                    nc.vector.tensor_scalar_add(out=den[:], in0=den[:],
                                                scalar1=1e-16)
                    if _stop(5):
                        continue
                    # msg = v_slot * w ; num = sum_j msg
                    kv_v = AP(kvt.tensor, kvt[:].offset + F,
                              [kvt[:].ap[0], [KVW, D], [1, F]])
                    w_b = AP(wt2.tensor, wt2[:].offset,
                             [wt2[:].ap[0], [1, D], [0, F]])
                    nc.vector.tensor_tensor(out=prod[:], in0=kv_v, in1=w_b,
                                            op=AluOpType.mult)
                    if _stop(6):
                        continue
                    num = sm.tile([P, F], FP, tag="num")
                    nc.vector.tensor_reduce(
                        out=num[:],
                        in_=AP(prod.tensor, prod[:].offset,
                               [prod[:].ap[0], [1, F], [F, D]]),
                        axis=AX, op=AluOpType.add)
                    if _stop(7):
                        continue
                    wa = sm.tile([P, 1], FP, tag="wa")
                    junk = sm.tile([P, D], FP, tag="junk")
                    nc.vector.tensor_tensor(out=junk[:], in0=wt2[:], in1=at[:],
                                            op=AluOpType.mult)
                    nc.vector.reduce_sum(out=wa[:], in_=junk[:], axis=AX)
                    nc.vector.scalar_tensor_tensor(
                        out=num[:], in0=weR_t[:, layer * F:(layer + 1) * F],
                        scalar=wa[:], in1=num[:],
                        op0=AluOpType.mult, op1=AluOpType.add)
                    if _stop(9):
                        continue
                    rec = sm.tile([P, 1], FP, tag="rec")
                    nc.vector.reciprocal(out=rec[:], in_=den[:])
                    h = sm.tile([P, F], FP, tag="h")
                    nc.vector.scalar_tensor_tensor(
                        out=h[:], in0=num[:], scalar=rec[:],
                        in1=qsw[:, F:2 * F], op0=AluOpType.mult,
                        op1=AluOpType.add)
                    nc.scalar.activation(out=h[:], in_=h[:], func=Act.Relu)
                    if BPART == 2:
                        continue

                    if layer == 0:
                        psT = psp.tile([F, P], FP, tag="psT")
                        nc.tensor.transpose(out=psT[:], in_=h[:],
                                            identity=ident[:])
                        hT = sm.tile([F, P], FP, tag="hT")
                        nc.vector.tensor_copy(out=hT[:], in_=psT[:])
                        nc.sync.dma_start(out=hTl[t, :, :], in_=hT[:])
                    else:
                        o3 = sm.tile([P, 3], FP, tag="o3")
                        fcj = sm.tile([P, F], FP, tag="fcj")
                        for cc in range(3):
                            nc.vector.tensor_tensor(
                                out=fcj[:], in0=h[:],
                                in1=wfc_t[:, cc * F:(cc + 1) * F],
                                op=AluOpType.mult)
                            nc.vector.reduce_sum(out=o3[:, cc:cc + 1],
                                                 in_=fcj[:], axis=AX)
                        nc.vector.tensor_add(out=o3[:], in0=o3[:],
                                             in1=bfc_t[:, :])
                        nrm2 = sm.tile([P, 1], FP, tag="nrm2")
                        j3 = sm.tile([P, 3], FP, tag="j3")
                        nc.vector.tensor_tensor(out=j3[:], in0=o3[:],
                                                in1=o3[:], op=AluOpType.mult)
                        nc.vector.reduce_sum(out=nrm2[:], in_=j3[:], axis=AX)
                        nrm = sm.tile([P, 1], FP, tag="nrm")
                        nc.scalar.activation(out=nrm[:], in_=nrm2[:],
                                             func=Act.Sqrt)
                        nc.vector.tensor_scalar(
                            out=nrm[:], in0=nrm[:], scalar1=1e-12,
                            scalar2=None, op0=AluOpType.max)
                        nc.vector.reciprocal(out=nrm[:], in_=nrm[:])
                        nc.vector.tensor_scalar(
                            out=o3[:], in0=o3[:], scalar1=nrm[:],
                            scalar2=10.0, op0=AluOpType.mult,
                            op1=AluOpType.mult)
                        xm = sm.tile([P, 2], FP, tag="xm")
                        nc.sync.dma_start(out=xm[:],
                                          in_=xmask[t * P:(t + 1) * P, :])
                        mk = sm.tile([P, 2], FP, tag="mk")
                        nc.vector.tensor_scalar(
                            out=mk[:, 0:1], in0=xm[:, 0:1], scalar1=-1.0,
                            scalar2=None, op0=AluOpType.is_equal)
                        nc.vector.tensor_scalar(
                            out=mk[:, 1:2], in0=xm[:, 1:2], scalar1=1.0,
                            scalar2=None, op0=AluOpType.is_equal)
                        nc.vector.scalar_tensor_tensor(
                            out=o3[:, 0:1], in0=mk[:, 0:1], scalar=-10.0,
                            in1=o3[:, 0:1], op0=AluOpType.mult,
                            op1=AluOpType.add)
                        nc.vector.scalar_tensor_tensor(
                            out=o3[:, 2:3], in0=mk[:, 1:2], scalar=-10.0,
                            in1=o3[:, 2:3], op0=AluOpType.mult,
                            op1=AluOpType.add)
                        nc.sync.dma_start(out=out3[t * P:(t + 1) * P, :],
                                          in_=o3[:])

        if MAXPH >= 2:
            edge_phase(kv1, qs1, 0)
        tc.strict_bb_all_engine_barrier()
        if MAXPH >= 3:
            nc.gpsimd.collective_compute(
            "AllGather", AluOpType.bypass,
            replica_groups=[list(range(NCORES))],
                ins=[hTl[:, :, :]],
                outs=[hTf[:, :, :]],
            )
        tc.strict_bb_all_engine_barrier()

        # ---------------- phase D: layer-2 projections ----------------
        with tc.tile_pool(name="pd_h", bufs=3) as pd_h, \
             tc.tile_pool(name="pd_ps", bufs=2, space="PSUM") as pd_ps, \
             tc.tile_pool(name="pd_sb", bufs=3) as pd_sb:
            for g in range(GT if MAXPH >= 4 else 0):
                ht = pd_h.tile([65, P], FP, tag="ht")
                nc.vector.memset(ht[64:65, :], 1.0)
                nc.sync.dma_start(out=ht[0:64, :], in_=hTf[g, :, :])
                ps = pd_ps.tile([P, 128], FP)
                nc.tensor.matmul(out=ps[:], lhsT=ht[:], rhs=w2kv_t[:],
                                 start=True, stop=True)
                sb = pd_sb.tile([P, 128], FP)
                nc.vector.tensor_copy(out=sb[:], in_=ps[:])
                nc.sync.dma_start(out=kv2[g * P:(g + 1) * P, :], in_=sb[:])
            for t in range(NT if MAXPH >= 4 else 0):
                ht = pd_h.tile([65, P], FP, tag="ht")
                nc.vector.memset(ht[64:65, :], 1.0)
                nc.sync.dma_start(out=ht[0:64, :], in_=hTl[t, :, :])
                ps = pd_ps.tile([P, 129], FP, tag="ps_qs")
                nc.tensor.matmul(out=ps[:], lhsT=ht[:], rhs=w2qs_t[:],
                                 start=True, stop=True)
                sb = pd_sb.tile([P, 129], FP, tag="sb_qs")
                nc.vector.tensor_copy(out=sb[:], in_=ps[:])
                nc.sync.dma_start(out=qs2[t, :, :], in_=sb[:])
        tc.strict_bb_all_engine_barrier()

        if MAXPH >= 5:
            edge_phase(kv2, qs2, 1)

    nc.compile()
    return nc


# ----------------------------------------------------------------------------
# Entry point
# ----------------------------------------------------------------------------

_CACHE = {}
LAST_EXEC_NS = None


def kernel(x, edge_index, edge_attr, params):
    x = np.asarray(x, np.float32)
    edge_index = np.asarray(edge_index, np.int32)
    edge_attr = np.asarray(edge_attr, np.float32)
    N = x.shape[0]

    H = _build_host(x, edge_index, edge_attr)
    W = _pack_weights(params, H, x)
    nc = _build_program(H)

    NT, NLOCP = H["NT"], H["NLOCP"]
    GT = H["NPOS"] // P
    in_maps = []
    bfc_rep = np.tile(W["bfc"][None, :], (P, 1)).astype(np.float32)
    for c in range(NCORES):
        cd = H["cores"][c]
        in_maps.append({
            "xT": W["xT"],
            "xTl": W["xT"][c * NT:(c + 1) * NT],
            "w1kv": W["w1kv"], "w1qs": W["w1qs"],
            "w2kv": W["w2kv"], "w2qs": W["w2qs"],
            "weR": W["weR"], "wfc": W["wfc"], "bfc": bfc_rep,
            "attr": cd["attr"], "valid": cd["valid"], "idx": cd["idx"],
            "xmask": W["xmask"][c],
        })

    global LAST_EXEC_NS
    trace = bool(int(__import__("os").environ.get("KERNEL_TRACE", "1")))
    try:
        if trace:
            res = run_bass_kernel_spmd(nc, in_maps,
                                       core_ids=list(range(NCORES)),
                                       trace=True, trace_cores=[0])
            LAST_EXEC_NS = res.exec_time_ns
        else:
            res = run_bass_kernel_spmd(nc, in_maps,
                                       core_ids=list(range(NCORES)))
    except Exception:
        if not trace:
            raise
        res = run_bass_kernel_spmd(nc, in_maps, core_ids=list(range(NCORES)))

    out = np.zeros((N, 3), np.float32)
    pos = H["pos_of"]
    full = np.concatenate([res.results[c]["out3"] for c in range(NCORES)], 0)
    out[:, :] = full.reshape(NCORES * NLOCP, 3)[pos]
    return out[:-1]


if __name__ == "__main__":
    pass
